# revision 7
# baseline (speedup 1.0000x reference)
"""BertSelfAttention Trainium2 Bass kernel.

B=8, S=1024, D=1024, H=16 heads, head_dim=64. Data-parallel: batch element b
runs on NeuronCore b (no collectives).

Per-core math (exact reference semantics, fp32 throughout):
  Q^T = Wq^T X^T + bq  (bias per-partition via ACT)   [d, q] layout
  K^T = Wk^T X^T + bk                                  [d, k] layout
  V   = X Wv + bv      (bias via K=1 ones-row matmul)  [k, d] layout, stored
        head-padded [k, 16*(64+1)] with a ones column per head
  per head h:
    scoresT[k, q] = sum_d K^T[d,k] Q^T[d,q]            (K=64 matmuls)
    expT = exp(scoresT/8 + mask[k])                    (ACT, per-partition bias)
    ctx_aug[q, 65] = sum_k expT[k,q] * [V_h | 1][k,:]  (accumulated over k)
    out[q, 64h:64h+64] = ctx_aug[:, :64] * (1/ctx_aug[:, 64])
  softmax needs no max-subtraction (scores ~ N(0,1); exp is fp32-safe) and the
  ones column yields the denominator inside the same PSUM accumulation.
"""

import sys

sys.path.insert(0, "/opt/trn_rl_repo")

import numpy as np

import concourse.bass as bass  # noqa: E402
import concourse.tile as tile  # noqa: E402
from concourse import bacc, mybir  # noqa: E402
from concourse.bass import ds, ts  # noqa: E402
from concourse.bass_utils import run_bass_kernel_spmd  # noqa: E402
from concourse.masks import make_identity  # noqa: E402

B, S, D, H = 8, 1024, 1024, 16
HD = D // H  # 64
P = 128
NCH = S // P  # 8
HP = HD + 1  # 65: head block incl. ones column
FP32 = mybir.dt.float32
EXP = mybir.ActivationFunctionType.Exp

_CACHED = {}


def _build_kernel(tc):
    nc = tc.nc
    x_d = nc.dram_tensor("x", [S, D], FP32, kind="ExternalInput").ap()
    mask_d = nc.dram_tensor("mask", [S], FP32, kind="ExternalInput").ap()
    wq_d = nc.dram_tensor("Wq", [D, D], FP32, kind="ExternalInput").ap()
    bq_d = nc.dram_tensor("bq", [D], FP32, kind="ExternalInput").ap()
    wk_d = nc.dram_tensor("Wk", [D, D], FP32, kind="ExternalInput").ap()
    bk_d = nc.dram_tensor("bk", [D], FP32, kind="ExternalInput").ap()
    wv_d = nc.dram_tensor("Wv", [D, D], FP32, kind="ExternalInput").ap()
    bv_d = nc.dram_tensor("bv", [D], FP32, kind="ExternalInput").ap()
    out_d = nc.dram_tensor("out", [S, D], FP32, kind="ExternalOutput").ap()

    with (
        tc.tile_pool(name="const", bufs=1) as const,
        tc.tile_pool(name="persist", bufs=1) as persist,
    ):
        identity = const.tile([P, P], FP32)
        make_identity(nc, identity[:])
        # per-partition vectors: v_sb[p, c] = vec[128c + p]
        mask_sb = const.tile([P, NCH], FP32)
        nc.sync.dma_start(out=mask_sb[:], in_=mask_d.rearrange("(c p) -> p c", p=P))
        bq_sb = const.tile([P, NCH], FP32)
        nc.sync.dma_start(out=bq_sb[:], in_=bq_d.rearrange("(c p) -> p c", p=P))
        bk_sb = const.tile([P, NCH], FP32)
        nc.sync.dma_start(out=bk_sb[:], in_=bk_d.rearrange("(c p) -> p c", p=P))
        bv_sb = const.tile([1, D], FP32)
        nc.sync.dma_start(out=bv_sb[:], in_=bv_d.rearrange("(a d) -> a d", a=1))
        ones_row = const.tile([1, P], FP32)
        nc.gpsimd.memset(ones_row[:], 1.0)

        qt_sb = persist.tile([P, NCH, S], FP32, tag="qt")  # Q^T: [d, q]
        kt_sb = persist.tile([P, NCH, S], FP32, tag="kt")  # K^T: [d, k]
        v_sb = persist.tile([P, NCH, H, HP], FP32, tag="v")  # V: [k, head-padded d]
        out_sb = persist.tile([P, NCH, D], FP32, tag="out")

        # ones columns for the softmax-denominator trick
        nc.gpsimd.memset(v_sb[:, :, :, HD : HD + 1], 1.0)

        # ---- phase 1: X^T via PE transposes ----
        with tc.tile_pool(name="xt", bufs=1) as xtp:
            xt_sb = xtp.tile([P, NCH, S], FP32, tag="xt")  # X^T: [c, s]
            with (
                tc.tile_pool(name="xpool", bufs=1) as xpool,
                tc.tile_pool(name="tpsum", bufs=2, space="PSUM") as tpsum,
            ):
                x_sb = xpool.tile([P, NCH, D], FP32, tag="x")
                for j in range(NCH):
                    nc.sync.dma_start(out=x_sb[:, j], in_=x_d[ts(j, P), :])
                for i in range(NCH):
                    for j in range(NCH):
                        pt = tpsum.tile([P, P], FP32, tag="tp")
                        nc.tensor.transpose(pt[:], x_sb[:, j, ts(i, P)], identity[:])
                        nc.vector.tensor_copy(out=xt_sb[:, i, ts(j, P)], in_=pt[:])

            # ---- phase 2: projections ----
            with (
                tc.tile_pool(name="wpool", bufs=2) as wpool,
                tc.tile_pool(name="ppsum", bufs=2, space="PSUM") as ppsum,
            ):
                for which in ("q", "k", "v"):
                    w_d = {"q": wq_d, "k": wk_d, "v": wv_d}[which]
                    w_half = []
                    for half in range(2):
                        wt = wpool.tile([P, NCH // 2, D], FP32, tag="w")
                        for k in range(NCH // 2):
                            nc.sync.dma_start(
                                out=wt[:, k], in_=w_d[ts(half * (NCH // 2) + k, P), :]
                            )
                        w_half.append(wt)

                    def w_chunk(k, cols):
                        return w_half[k // 4][:, k % 4, cols]

                    for c in range(NCH):
                        pt = ppsum.tile([P, S], FP32, tag="proj")
                        for n in range(2):
                            po = pt[:, ts(n, 512)]
                            for k in range(NCH):
                                if which == "v":
                                    # V[s,d]: lhsT = X^T chunk [c', s], rhs = Wv [c', d]
                                    lhsT = xt_sb[:, k, ts(c, P)]
                                    rhs = w_chunk(k, ts(n, 512))
                                else:
                                    # Q^T/K^T [d,*]: lhsT = W chunk [c', d], rhs = X^T
                                    lhsT = w_chunk(k, ts(c, P))
                                    rhs = xt_sb[:, k, ts(n, 512)]
                                nc.tensor.matmul(
                                    out=po,
                                    lhsT=lhsT,
                                    rhs=rhs,
                                    start=(k == 0),
                                    stop=(k == NCH - 1 and which != "v"),
                                )
                            if which == "v":  # += ones^T @ bv  (adds bias along d)
                                nc.tensor.matmul(
                                    out=po,
                                    lhsT=ones_row[:],
                                    rhs=bv_sb[:, ts(n, 512)],
                                    start=False,
                                    stop=True,
                                )
                            # evacuate PSUM -> SBUF
                            if which == "q":
                                nc.scalar.add(
                                    out=qt_sb[:, c, ts(n, 512)], in_=po, add=bq_sb[:, c : c + 1]
                                )
                            elif which == "k":
                                nc.scalar.add(
                                    out=kt_sb[:, c, ts(n, 512)], in_=po, add=bk_sb[:, c : c + 1]
                                )
                            else:
                                nc.vector.tensor_copy(
                                    out=v_sb[:, c, ds(8 * n, 8), 0:HD],
                                    in_=po.rearrange("p (h d) -> p h d", d=HD),
                                )

        # ---- phase 3: attention per head ----
        with (
            tc.tile_pool(name="exppool", bufs=2) as exppool,
            tc.tile_pool(name="rnpool", bufs=8) as rnpool,
            tc.tile_pool(name="spsum", bufs=2, space="PSUM") as spsum,
            tc.tile_pool(name="cpsum", bufs=4, space="PSUM") as cpsum,
        ):
            for h in range(H):
                ch, oh = h // 2, HD * (h % 2)
                expT = exppool.tile([P, NCH, S], FP32, tag="exp")
                for i in range(NCH):
                    sp = spsum.tile([P, S], FP32, tag="scores")
                    for n in range(2):
                        nc.tensor.matmul(
                            out=sp[:, ts(n, 512)],
                            lhsT=kt_sb[oh : oh + HD, ch, ts(i, P)],
                            rhs=qt_sb[oh : oh + HD, ch, ts(n, 512)],
                            start=True,
                            stop=True,
                        )
                        nc.scalar.activation(
                            out=expT[:, i, ts(n, 512)],
                            in_=sp[:, ts(n, 512)],
                            func=EXP,
                            bias=mask_sb[:, i : i + 1],
                            scale=1.0 / np.sqrt(HD).item(),
                        )
                for j in range(NCH):
                    ctxp = cpsum.tile([P, HP], FP32, tag="ctx")
                    for i in range(NCH):
                        nc.tensor.matmul(
                            out=ctxp[:],
                            lhsT=expT[:, i, ts(j, P)],
                            rhs=v_sb[:, i, h, :],
                            start=(i == 0),
                            stop=(i == NCH - 1),
                        )
                    rn = rnpool.tile([P, 1], FP32, tag="rn")
                    nc.vector.reciprocal(rn[:], ctxp[:, HD : HD + 1])
                    nc.vector.tensor_scalar_mul(
                        out_sb[:, j, ds(HD * h, HD)], ctxp[:, 0:HD], rn[:]
                    )

        for j in range(NCH):
            nc.sync.dma_start(out=out_d[ts(j, P), :], in_=out_sb[:, j])


def _ensure_ntff_hook():
    """antenv.axon_hooks is absent in this image; recreate it so
    run_bass_kernel_spmd(trace=True) can capture NTFF profiles."""
    import types

    try:
        from antenv.axon_hooks import get_axon_ntff_profile_hook  # noqa: F401

        return
    except ImportError:
        pass
    from trn_agent_boot.trn_boot import _ntff_profile_via_ctypes

    hook = _ntff_profile_via_ctypes("/opt/axon/libaxon_pjrt.so")
    mod = types.ModuleType("antenv.axon_hooks")
    mod._hook = hook
    mod.get_axon_ntff_profile_hook = lambda: mod._hook
    mod.set_axon_ntff_profile_hook = lambda h: setattr(mod, "_hook", h)
    sys.modules["antenv.axon_hooks"] = mod


def _get_compiled():
    if "nc" not in _CACHED:
        nc = bacc.Bacc(
            "TRN2", target_bir_lowering=False, debug=False, num_devices=B
        )
        with tile.TileContext(nc) as tc:
            _build_kernel(tc)
        nc.compile()
        _CACHED["nc"] = nc
    return _CACHED["nc"]


def kernel(hidden_states, attention_mask, Wq, bq, Wk, bk, Wv, bv, **run_kwargs):
    hs = np.ascontiguousarray(np.asarray(hidden_states, dtype=np.float32))
    am = np.ascontiguousarray(np.asarray(attention_mask, dtype=np.float32)).reshape(B, S)
    weights = {
        "Wq": np.ascontiguousarray(np.asarray(Wq, dtype=np.float32)),
        "bq": np.ascontiguousarray(np.asarray(bq, dtype=np.float32)),
        "Wk": np.ascontiguousarray(np.asarray(Wk, dtype=np.float32)),
        "bk": np.ascontiguousarray(np.asarray(bk, dtype=np.float32)),
        "Wv": np.ascontiguousarray(np.asarray(Wv, dtype=np.float32)),
        "bv": np.ascontiguousarray(np.asarray(bv, dtype=np.float32)),
    }
    if run_kwargs.get("trace"):
        _ensure_ntff_hook()
    nc = _get_compiled()
    in_maps = [
        {"x": hs[b], "mask": am[b], **weights} for b in range(B)
    ]
    res = run_bass_kernel_spmd(nc, in_maps, core_ids=list(range(B)), **run_kwargs)
    out = np.stack([res.results[b]["out"] for b in range(B)], axis=0)
    if run_kwargs:
        kernel.last_results = res
    return out


if __name__ == "__main__":
    rng = np.random.default_rng(0)
    inputs = {
        "hidden_states": rng.standard_normal((B, S, D), dtype=np.float32),
        "attention_mask": np.zeros((B, 1, 1, S), dtype=np.float32),
        "Wq": rng.standard_normal((D, D), dtype=np.float32) / 32.0,
        "bq": rng.standard_normal(D, dtype=np.float32) * 0.02,
        "Wk": rng.standard_normal((D, D), dtype=np.float32) / 32.0,
        "bk": rng.standard_normal(D, dtype=np.float32) * 0.02,
        "Wv": rng.standard_normal((D, D), dtype=np.float32) / 32.0,
        "bv": rng.standard_normal(D, dtype=np.float32) * 0.02,
    }
    out = kernel(**inputs)
    print("out", out.shape, out.dtype, float(np.abs(out).mean()))


# revision 12
# speedup vs baseline: 2.0699x; 2.0699x over previous
"""BertSelfAttention Trainium2 Bass kernel.

B=8, S=1024, D=1024, H=16 heads, head_dim=64. Data-parallel: batch element b
runs on NeuronCore b (no collectives).

Per-core math (exact reference semantics, fp32 throughout):
  Q^T = Wq^T X^T + bq  (bias per-partition via ACT)   [d, q] layout
  K^T = Wk^T X^T + bk                                  [d, k] layout
  V   = X Wv + bv      (bias via K=1 ones-row matmul)  [k, d] layout, stored
        head-padded [k, 16*(64+1)] with a ones column per head
  per head h:
    scoresT[k, q] = sum_d K^T[d,k] Q^T[d,q]            (K=64 matmuls)
    expT = exp(scoresT/8 + mask[k])                    (ACT, per-partition bias)
    ctx_aug[q, 65] = sum_k expT[k,q] * [V_h | 1][k,:]  (accumulated over k)
    out[q, 64h:64h+64] = ctx_aug[:, :64] * (1/ctx_aug[:, 64])
  softmax needs no max-subtraction (scores ~ N(0,1); exp is fp32-safe) and the
  ones column yields the denominator inside the same PSUM accumulation.
"""

import sys

sys.path.insert(0, "/opt/trn_rl_repo")

import numpy as np

import concourse.bass as bass  # noqa: E402
import concourse.tile as tile  # noqa: E402
from concourse import bacc, mybir  # noqa: E402
from concourse.bass import ds, ts  # noqa: E402
from concourse.bass_utils import run_bass_kernel_spmd  # noqa: E402
from concourse.masks import make_identity  # noqa: E402

B, S, D, H = 8, 1024, 1024, 16
HD = D // H  # 64
P = 128
NCH = S // P  # 8
HP = HD + 2  # 66: head block incl. ones column (+pad; fp32r needs even N)
FP32 = mybir.dt.float32
FP32R = mybir.dt.float32r
USE_FP32R = True
MMDT = FP32R if USE_FP32R else FP32
EXP = mybir.ActivationFunctionType.Exp


def _mm(nc, out, lhsT, rhs, start, stop):
    nc.tensor.matmul(out=out, lhsT=lhsT, rhs=rhs, start=start, stop=stop)

_CACHED = {}


def _build_kernel(tc):
    nc = tc.nc
    x_d = nc.dram_tensor("x", [S, D], FP32, kind="ExternalInput").ap()
    mask_d = nc.dram_tensor("mask", [S], FP32, kind="ExternalInput").ap()
    wq_d = nc.dram_tensor("Wq", [D, D], MMDT, kind="ExternalInput").ap()
    bq_d = nc.dram_tensor("bq", [D], FP32, kind="ExternalInput").ap()
    wk_d = nc.dram_tensor("Wk", [D, D], MMDT, kind="ExternalInput").ap()
    bk_d = nc.dram_tensor("bk", [D], FP32, kind="ExternalInput").ap()
    wv_d = nc.dram_tensor("Wv", [D, D], MMDT, kind="ExternalInput").ap()
    bv_d = nc.dram_tensor("bv", [D], MMDT, kind="ExternalInput").ap()
    out_d = nc.dram_tensor("out", [S, D], FP32, kind="ExternalOutput").ap()

    with (
        tc.tile_pool(name="const", bufs=1) as const,
        tc.tile_pool(name="persist", bufs=1) as persist,
    ):
        identity = const.tile([P, P], FP32)
        make_identity(nc, identity[:])
        # per-partition vectors: v_sb[p, c] = vec[128c + p]
        mask_sb = const.tile([P, NCH], FP32)
        nc.sync.dma_start(out=mask_sb[:], in_=mask_d.rearrange("(c p) -> p c", p=P))
        bq_sb = const.tile([P, NCH], FP32)
        nc.sync.dma_start(out=bq_sb[:], in_=bq_d.rearrange("(c p) -> p c", p=P))
        bk_sb = const.tile([P, NCH], FP32)
        nc.sync.dma_start(out=bk_sb[:], in_=bk_d.rearrange("(c p) -> p c", p=P))
        bv_sb = const.tile([1, D], MMDT)
        nc.sync.dma_start(out=bv_sb[:], in_=bv_d.rearrange("(a d) -> a d", a=1))
        ones_row = const.tile([1, P], MMDT)
        nc.gpsimd.memset(ones_row[:].bitcast(mybir.dt.uint32), 0x3F800000)

        qt_sb = persist.tile([P, NCH, S], MMDT, tag="qt")  # Q^T: [d, q]
        kt_sb = persist.tile([P, NCH, S], MMDT, tag="kt")  # K^T: [d, k]
        v_sb = persist.tile([P, NCH, H, HP], MMDT, tag="v")  # V: [k, head-padded d]
        out_sb = persist.tile([P, NCH, D], FP32, tag="out")

        # ones columns for the softmax-denominator trick
        nc.gpsimd.memset(v_sb[:, :, :, HD : HD + 2].bitcast(mybir.dt.uint32), 0x3F800000)

        # ---- phase 1: X^T via PE transposes ----
        with tc.tile_pool(name="xt", bufs=1) as xtp:
            xt_sb = xtp.tile([P, NCH, S], MMDT, tag="xt")  # X^T: [c, s]
            with (
                tc.tile_pool(name="xpool", bufs=1) as xpool,
                tc.tile_pool(name="tpsum", bufs=2, space="PSUM") as tpsum,
            ):
                x_sb = xpool.tile([P, NCH, D], FP32, tag="x")
                for j in range(NCH):
                    nc.sync.dma_start(out=x_sb[:, j], in_=x_d[ts(j, P), :])
                for i in range(NCH):
                    for j in range(NCH):
                        pt = tpsum.tile([P, P], FP32, tag="tp")
                        nc.tensor.transpose(pt[:], x_sb[:, j, ts(i, P)], identity[:])
                        nc.vector.tensor_copy(out=xt_sb[:, i, ts(j, P)], in_=pt[:])

            # ---- phase 2: projections ----
            with (
                tc.tile_pool(name="wpool", bufs=2) as wpool,
                tc.tile_pool(name="ppsum", bufs=2, space="PSUM") as ppsum,
            ):
                for which in ("q", "k", "v"):
                    w_d = {"q": wq_d, "k": wk_d, "v": wv_d}[which]
                    w_half = []
                    for half in range(2):
                        wt = wpool.tile([P, NCH // 2, D], MMDT, tag="w")
                        for k in range(NCH // 2):
                            nc.sync.dma_start(
                                out=wt[:, k], in_=w_d[ts(half * (NCH // 2) + k, P), :]
                            )
                        w_half.append(wt)

                    def w_chunk(k, cols):
                        return w_half[k // 4][:, k % 4, cols]

                    for c in range(NCH):
                        pt = ppsum.tile([P, S], FP32, tag="proj")
                        for n in range(2):
                            po = pt[:, ts(n, 512)]
                            for k in range(NCH):
                                if which == "v":
                                    # V[s,d]: lhsT = X^T chunk [c', s], rhs = Wv [c', d]
                                    lhsT = xt_sb[:, k, ts(c, P)]
                                    rhs = w_chunk(k, ts(n, 512))
                                else:
                                    # Q^T/K^T [d,*]: lhsT = W chunk [c', d], rhs = X^T
                                    lhsT = w_chunk(k, ts(c, P))
                                    rhs = xt_sb[:, k, ts(n, 512)]
                                _mm(nc, po, lhsT, rhs,
                                    (k == 0), (k == NCH - 1 and which != "v"))
                            if which == "v":  # += ones^T @ bv  (adds bias along d)
                                _mm(nc, po, ones_row[:], bv_sb[:, ts(n, 512)], False, True)
                            # evacuate PSUM -> SBUF
                            if which == "q":
                                nc.scalar.add(
                                    out=qt_sb[:, c, ts(n, 512)], in_=po, add=bq_sb[:, c : c + 1]
                                )
                            elif which == "k":
                                nc.scalar.add(
                                    out=kt_sb[:, c, ts(n, 512)], in_=po, add=bk_sb[:, c : c + 1]
                                )
                            else:
                                nc.vector.tensor_copy(
                                    out=v_sb[:, c, ds(8 * n, 8), 0:HD],
                                    in_=po.rearrange("p (h d) -> p h d", d=HD),
                                )

        # ---- phase 3: attention per head ----
        with (
            tc.tile_pool(name="exppool", bufs=2) as exppool,
            tc.tile_pool(name="rnpool", bufs=8) as rnpool,
            tc.tile_pool(name="spsum", bufs=2, space="PSUM") as spsum,
            tc.tile_pool(name="cpsum", bufs=4, space="PSUM") as cpsum,
        ):
            for h in range(H):
                ch, oh = h // 2, HD * (h % 2)
                expT = exppool.tile([P, NCH, S], MMDT, tag="exp")
                for i in range(NCH):
                    sp = spsum.tile([P, S], FP32, tag="scores")
                    for n in range(2):
                        _mm(nc, sp[:, ts(n, 512)],
                            kt_sb[oh : oh + HD, ch, ts(i, P)],
                            qt_sb[oh : oh + HD, ch, ts(n, 512)], True, True)
                        nc.scalar.activation(
                            out=expT[:, i, ts(n, 512)],
                            in_=sp[:, ts(n, 512)],
                            func=EXP,
                            bias=mask_sb[:, i : i + 1],
                            scale=1.0 / np.sqrt(HD).item(),
                        )
                for j in range(NCH):
                    ctxp = cpsum.tile([P, HP], FP32, tag="ctx")
                    for i in range(NCH):
                        _mm(nc, ctxp[:], expT[:, i, ts(j, P)], v_sb[:, i, h, :],
                            (i == 0), (i == NCH - 1))
                    rn = rnpool.tile([P, 1], FP32, tag="rn")
                    nc.vector.reciprocal(rn[:], ctxp[:, HD : HD + 1])
                    nc.vector.tensor_scalar_mul(
                        out_sb[:, j, ds(HD * h, HD)], ctxp[:, 0:HD], rn[:]
                    )

        for j in range(NCH):
            nc.sync.dma_start(out=out_d[ts(j, P), :], in_=out_sb[:, j])


def _ensure_ntff_hook():
    """antenv.axon_hooks is absent in this image; recreate it so
    run_bass_kernel_spmd(trace=True) can capture NTFF profiles."""
    import types

    try:
        from antenv.axon_hooks import get_axon_ntff_profile_hook  # noqa: F401

        return
    except ImportError:
        pass
    from trn_agent_boot.trn_boot import _ntff_profile_via_ctypes

    hook = _ntff_profile_via_ctypes("/opt/axon/libaxon_pjrt.so")
    mod = types.ModuleType("antenv.axon_hooks")
    mod._hook = hook
    mod.get_axon_ntff_profile_hook = lambda: mod._hook
    mod.set_axon_ntff_profile_hook = lambda h: setattr(mod, "_hook", h)
    sys.modules["antenv.axon_hooks"] = mod


def _get_compiled():
    if "nc" not in _CACHED:
        nc = bacc.Bacc(
            "TRN2", target_bir_lowering=False, debug=False, num_devices=B
        )
        with tile.TileContext(nc) as tc:
            _build_kernel(tc)
        nc.compile()
        _CACHED["nc"] = nc
    return _CACHED["nc"]


def kernel(hidden_states, attention_mask, Wq, bq, Wk, bk, Wv, bv, **run_kwargs):
    hs = np.ascontiguousarray(np.asarray(hidden_states, dtype=np.float32))
    am = np.ascontiguousarray(np.asarray(attention_mask, dtype=np.float32)).reshape(B, S)
    weights = {
        "Wq": np.ascontiguousarray(np.asarray(Wq, dtype=np.float32)),
        "bq": np.ascontiguousarray(np.asarray(bq, dtype=np.float32)),
        "Wk": np.ascontiguousarray(np.asarray(Wk, dtype=np.float32)),
        "bk": np.ascontiguousarray(np.asarray(bk, dtype=np.float32)),
        "Wv": np.ascontiguousarray(np.asarray(Wv, dtype=np.float32)),
        "bv": np.ascontiguousarray(np.asarray(bv, dtype=np.float32)),
    }
    if run_kwargs.get("trace"):
        _ensure_ntff_hook()
    nc = _get_compiled()
    in_maps = [
        {"x": hs[b], "mask": am[b], **weights} for b in range(B)
    ]
    res = run_bass_kernel_spmd(nc, in_maps, core_ids=list(range(B)), **run_kwargs)
    out = np.stack([res.results[b]["out"] for b in range(B)], axis=0)
    if run_kwargs:
        kernel.last_results = res
    return out


if __name__ == "__main__":
    rng = np.random.default_rng(0)
    inputs = {
        "hidden_states": rng.standard_normal((B, S, D), dtype=np.float32),
        "attention_mask": np.zeros((B, 1, 1, S), dtype=np.float32),
        "Wq": rng.standard_normal((D, D), dtype=np.float32) / 32.0,
        "bq": rng.standard_normal(D, dtype=np.float32) * 0.02,
        "Wk": rng.standard_normal((D, D), dtype=np.float32) / 32.0,
        "bk": rng.standard_normal(D, dtype=np.float32) * 0.02,
        "Wv": rng.standard_normal((D, D), dtype=np.float32) / 32.0,
        "bv": rng.standard_normal(D, dtype=np.float32) * 0.02,
    }
    out = kernel(**inputs)
    print("out", out.shape, out.dtype, float(np.abs(out).mean()))
